# revision 1
# baseline (speedup 1.0000x reference)
"""Multi-head attention (qkv proj + softmax attention + out proj) on 8 trn2 cores.

Sharding: zero-collective. Core c handles batch b=c//2 and query-half h=c%2
(1024 queries). The host passes x[b] ROTATED by h*1024 tokens so that each
core's queries are always local tokens 0..1023 while K/V still cover all 2048
tokens (key order is irrelevant to softmax). Host concatenates the 8 output
slabs [1024, 1024] into [4, 2048, 1024].

Per-core kernel (all on one NeuronCore, Tile-scheduled):
  1. PE-transpose x and the weights to contraction-major layout (fp32 in,
     bf16 out via the DVE PSUM-evacuation copy).
  2. QKV projection in bf16: QT/KT stored feature-major [d, t] so each
     128-row tile holds a PAIR of heads (2 x 64 d-rows); V stored token-major
     with interleaved ones columns (fused softmax-sum rows come out of the
     AV matmul for free).
  3. Attention per head-pair: QK^T into PSUM, softmax WITHOUT
     max-subtraction (scores are ~N(0,1); fp32 exp is safe), exp on ScalarE
     straight out of PSUM, AV+sums fused, fast approximate reciprocal on DVE.
  4. Out-projection in bf16 + bias, fp32 result.
"""

import numpy as np

B, N, C = 4, 2048, 1024
H, D = 16, 64
P = 128
CG = C // P            # 8 contraction groups
TG = N // P            # 16 key-token chunks
TQ = N // 2            # 1024 queries per core
QB = 512               # query block (psum bank)
NB = TQ // QB          # 2
NPAIR = H // 2         # 8 head pairs
SCALE = 1.0 / np.sqrt(D).astype(np.float32)
DEBUG_DUMPS = False

_CACHE = {}


def _build():
    import concourse.tile as tile
    from concourse import bacc, mybir

    f32 = mybir.dt.float32
    nc = bacc.Bacc(
        "TRN2", target_bir_lowering=False, debug=False, num_devices=8
    )
    x_h = nc.dram_tensor("x", [N, C], f32, kind="ExternalInput").ap()
    wqkv_h = nc.dram_tensor("w_qkv", [3 * C, C], f32, kind="ExternalInput").ap()
    wout_h = nc.dram_tensor("w_out", [C, C], f32, kind="ExternalInput").ap()
    bout_h = nc.dram_tensor("b_out", [C], f32, kind="ExternalInput").ap()
    y_h = nc.dram_tensor("y", [TQ, C], f32, kind="ExternalOutput").ap()

    with tile.TileContext(nc) as tc:
        _emit(tc, x_h, wqkv_h, wout_h, bout_h, y_h)
    nc.compile()
    return nc


def _emit(tc, x_h, wqkv_h, wout_h, bout_h, y_h):
    from contextlib import ExitStack

    from concourse import mybir
    from concourse.masks import make_identity

    f32 = mybir.dt.float32
    bf16 = mybir.dt.bfloat16
    AF = mybir.ActivationFunctionType
    nc = tc.nc

    with ExitStack() as ctx:
        # ---------------- pools ----------------
        const = ctx.enter_context(tc.tile_pool(name="const", bufs=1))
        big = ctx.enter_context(tc.tile_pool(name="big", bufs=1))
        land = ctx.enter_context(tc.tile_pool(name="land", bufs=3))
        cstp = ctx.enter_context(tc.tile_pool(name="cstp", bufs=5))
        ktp = ctx.enter_context(tc.tile_pool(name="ktp", bufs=2))
        qtp = ctx.enter_context(tc.tile_pool(name="qtp", bufs=2))
        wtq = ctx.enter_context(tc.tile_pool(name="wtq", bufs=2))
        wtv = ctx.enter_context(tc.tile_pool(name="wtv", bufs=2))
        ptp = ctx.enter_context(tc.tile_pool(name="ptp", bufs=17))
        recp = ctx.enter_context(tc.tile_pool(name="recp", bufs=2))
        outp = ctx.enter_context(tc.tile_pool(name="outp", bufs=2))
        ps_pool = ctx.enter_context(
            tc.tile_pool(name="ps_pool", bufs=2, space="PSUM")
        )
        st_ps = ctx.enter_context(
            tc.tile_pool(name="st_ps", bufs=2, space="PSUM")
        )
        at_ps = ctx.enter_context(
            tc.tile_pool(name="at_ps", bufs=2, space="PSUM")
        )

        # ---------------- constants ----------------
        ident = const.tile([P, P], bf16)
        make_identity(nc, ident)
        onesrow = const.tile([1, P], f32)
        nc.gpsimd.memset(onesrow, 1.0)

        # ---------------- persistent tensors ----------------
        # V layout per pair p: cols [192p,192p+64)=V_h2p, [192p+64,+128)=ones,
        # [192p+128,+192)=V_h2p+1. The AV stationary operand for head A is
        # cols [192p,192p+128) = [V_A | ones] (output rows 0:64 = attention
        # out, rows 64:128 = softmax sums, replicated); for head B it is cols
        # [192p+64,192p+192) = [ones | V_B] (sums on rows 0:64, attention out
        # on rows 64:128). The ones block is shared between the two heads.
        xT = [big.tile([P, N], bf16, name=f"xT{g}") for g in range(CG)]
        V = [big.tile([P, 192 * NPAIR], bf16, name=f"V{i}") for i in range(TG)]
        aoT = [big.tile([P, TQ], bf16, name=f"aoT{g}") for g in range(CG)]

        # -------- load x (fp32), cast bf16, PE-transpose into xT --------
        def load_cast(src_ap, name, scale=None):
            t = land.tile([P, C], f32, tag="land", name=f"ld{name}")
            nc.sync.dma_start(t, src_ap)
            b = cstp.tile([P, C], bf16, tag="cst", name=f"cs{name}")
            if scale is None:
                nc.vector.tensor_copy(b, t)
            else:
                nc.vector.tensor_scalar_mul(b, t, scale)
            return b

        xb16 = []
        for i in range(TG):
            xb16.append(load_cast(x_h[i * P : (i + 1) * P, :], f"x{i}"))

        # 4 [128,128] bf16 PE transposes into one psum bank, one copy.
        for i4 in range(0, TG, 4):
            for g in range(CG):
                psf = ps_pool.tile([P, QB], f32, tag="ps")
                ps = psf[:, :].bitcast(bf16)[:, 0 : 4 * P]
                for k in range(4):
                    nc.tensor.transpose(
                        ps[:, k * P : (k + 1) * P],
                        xb16[i4 + k][:, g * P : (g + 1) * P],
                        ident,
                    )
                nc.vector.tensor_copy(xT[g][:, i4 * P : (i4 + 4) * P], ps)

        def w_chunk_T(src_h, row0, pool, tag):
            """Load+transpose 4 consecutive 128-row chunks of a [*, C] fp32
            weight into a [P, CG, 512] bf16 tile (contraction-major)."""
            wt = pool.tile([P, CG, 4 * P], bf16, tag=tag)
            wn = []
            for jj in range(4):
                wn.append(
                    load_cast(
                        src_h[(row0 + jj) * P : (row0 + jj + 1) * P, :],
                        f"w{row0}_{jj}",
                    )
                )
            for g in range(CG):
                psf = ps_pool.tile([P, QB], f32, tag="ps")
                ps = psf[:, :].bitcast(bf16)[:, 0 : 4 * P]
                for jj in range(4):
                    nc.tensor.transpose(
                        ps[:, jj * P : (jj + 1) * P],
                        wn[jj][:, g * P : (g + 1) * P],
                        ident,
                    )
                nc.vector.tensor_copy(wt[:, g, :], ps)
            return wt

        # V ones columns: only needed before the first AV read (~40us in);
        # emitted here (after x) so they don't block make_identity or the
        # startup DMA chain on the in-order GpSimd queue.
        for i in range(TG):
            v3i = V[i].rearrange("p (q e) -> p q e", e=64)
            nc.gpsimd.memset(v3i[:, 1 : 3 * NPAIR : 3], 1.0)

        # ---- deferred-emission machinery: projection work for pair p+1 is
        # emitted in small quanta INTO pair p's attention emission, so the
        # Tile scheduler gives it priorities that interleave it into the
        # PE-idle slots of the (Scalar-exp-paced) attention phase instead of
        # serializing it at the pair boundary.
        def w_transp_gen(wn, wt):
            """Transpose a cast [P, C] weight tile into wt [P, CG, P]."""
            for g4 in range(0, CG, 4):
                psf = ps_pool.tile([P, QB], f32, tag="ps")
                ps = psf[:, :].bitcast(bf16)[:, 0 : 4 * P]
                for k in range(4):
                    nc.tensor.transpose(
                        ps[:, k * P : (k + 1) * P],
                        wn[:, (g4 + k) * P : (g4 + k + 1) * P],
                        ident,
                    )
                nc.vector.tensor_copy(wt[:, g4 : g4 + 4, :], ps)
                yield

        def kq_proj_gen(p, out, wn_k=None, wn_q=None):
            # K chunk p: w_qkv rows [C + p*128, C + (p+1)*128)
            KT = ktp.tile([P, N], bf16, tag="KT")
            wt = wtq.tile([P, CG, P], bf16, tag="wtq")
            wn = wn_k or load_cast(
                wqkv_h[(8 + p) * P : (9 + p) * P, :], f"wk{p}"
            )
            yield
            yield from w_transp_gen(wn, wt)
            for tbp in range(0, 4, 2):
                pss = [
                    ps_pool.tile([P, QB], f32, tag="ps", name=f"pjk{k}")
                    for k in range(2)
                ]
                for g in range(CG):
                    for k in range(2):
                        nc.tensor.matmul(
                            pss[k],
                            wt[:, g, :],
                            xT[g][:, (tbp + k) * QB : (tbp + k + 1) * QB],
                            start=(g == 0),
                            stop=(g == CG - 1),
                        )
                    yield
                for k in range(2):
                    nc.vector.tensor_copy(
                        KT[:, (tbp + k) * QB : (tbp + k + 1) * QB], pss[k]
                    )
                yield
            out["KT"] = KT

            # Q chunk p: w_qkv rows [p*128, (p+1)*128); queries are tokens
            # 0..TQ-1 of the (rotated) local x. Scale 1/sqrt(D) folded into
            # the psum evacuation copy.
            QT = qtp.tile([P, TQ], bf16, tag="QT")
            wt = wtq.tile([P, CG, P], bf16, tag="wtq")
            wn = wn_q or load_cast(
                wqkv_h[p * P : (p + 1) * P, :], f"wq{p}", scale=float(SCALE)
            )
            yield
            yield from w_transp_gen(wn, wt)
            pss = [
                ps_pool.tile([P, QB], f32, tag="ps", name=f"pjq{k}")
                for k in range(2)
            ]
            for g in range(CG):
                for k in range(2):
                    nc.tensor.matmul(
                        pss[k],
                        wt[:, g, :],
                        xT[g][:, k * QB : (k + 1) * QB],
                        start=(g == 0),
                        stop=(g == CG - 1),
                    )
                yield
            for k in range(2):
                nc.vector.tensor_copy(QT[:, k * QB : (k + 1) * QB], pss[k])
            out["QT"] = QT

        def w_chunk_T_gen(src_h, row0, pool, tag):
            wt = pool.tile([P, CG, 4 * P], bf16, tag=tag)
            wn = []
            for jj in range(4):
                wn.append(
                    load_cast(
                        src_h[(row0 + jj) * P : (row0 + jj + 1) * P, :],
                        f"w{row0}_{jj}",
                    )
                )
                yield
            for g in range(CG):
                psf = ps_pool.tile([P, QB], f32, tag="ps")
                ps = psf[:, :].bitcast(bf16)[:, 0 : 4 * P]
                for jj in range(4):
                    nc.tensor.transpose(
                        ps[:, jj * P : (jj + 1) * P],
                        wn[jj][:, g * P : (g + 1) * P],
                        ident,
                    )
                nc.vector.tensor_copy(wt[:, g, :], ps)
                yield
            return wt

        def v_proj_gen(fh, wvT=None):
            # V feature half fh: heads 8fh..8fh+7 (pairs 4fh..4fh+3).
            if wvT is None:
                wvT = yield from w_chunk_T_gen(wqkv_h, 16 + 4 * fh, wtv, "wtv")
            for i in range(TG):
                ps = ps_pool.tile([P, QB], f32, tag="ps")
                for g in range(CG):
                    nc.tensor.matmul(
                        ps,
                        xT[g][:, i * P : (i + 1) * P],
                        wvT[:, g, :],
                        start=(g == 0),
                        stop=(g == CG - 1),
                    )
                    if g % 4 == 3:
                        yield
                # psum cols = v features [512*fh, 512*(fh+1)) = heads
                # 8fh..8fh+7. Scatter per-head 64-col blocks into the
                # interleaved V layout: head h -> col 192*(h//2)+128*(h%2).
                ps3 = ps.rearrange("p (k e) -> p k e", e=64)
                v3 = V[i].rearrange("p (q e) -> p q e", e=64)
                b0 = 12 * fh
                nc.vector.tensor_copy(v3[:, b0 : b0 + 12 : 3], ps3[:, 0::2])
                nc.vector.tensor_copy(
                    v3[:, b0 + 2 : b0 + 12 : 3], ps3[:, 1::2]
                )
                yield

        def bias_woT_gen(out):
            bias = big.tile([P, C], f32, name="bias")
            bl = const.tile([1, C], f32)
            nc.gpsimd.dma_start(bl, bout_h.unsqueeze(0))
            yield
            for hh in range(2):
                ps = ps_pool.tile([P, QB], f32, tag="ps")
                nc.tensor.matmul(
                    ps, onesrow, bl[0:1, hh * QB : (hh + 1) * QB]
                )
                nc.scalar.copy(bias[:, hh * QB : (hh + 1) * QB], ps)
                yield
            out["bias"] = bias
            out["woT0"] = yield from w_chunk_T_gen(wout_h, 0, wtv, "wtv")
            out["woT1"] = yield from w_chunk_T_gen(wout_h, 4, wtv, "wtv")

        def out_proj_gen(i0, i1, ow):
            for i in range(i0, i1):
                ob = outp.tile([P, C], f32, tag="ob")
                for oh, woT in ((0, ow["woT0"]), (1, ow["woT1"])):
                    ps = ps_pool.tile([P, QB], f32, tag="ps")
                    for g in range(CG):
                        nc.tensor.matmul(
                            ps,
                            aoT[g][:, i * P : (i + 1) * P],
                            woT[:, g, :],
                            start=(g == 0),
                            stop=(g == CG - 1),
                        )
                        if g % 4 == 3:
                            yield
                    nc.vector.tensor_add(
                        ob[:, oh * QB : (oh + 1) * QB],
                        ps,
                        ow["bias"][:, oh * QB : (oh + 1) * QB],
                    )
                nc.sync.dma_start(y_h[i * P : (i + 1) * P, :], ob)
                yield

        pending = []

        def pump(n=1):
            for _ in range(n):
                while pending:
                    try:
                        next(pending[0])
                        break
                    except StopIteration:
                        pending.pop(0)
                else:
                    return

        def drain():
            while pending:
                pump()

        # ---- lagged AV: each block's AV + normalize is emitted chunk-by-
        # chunk INSIDE the next block's QK loop, so the Scalar exp chain
        # never stalls behind an AV wall at block/pair boundaries.
        def emit_av_chunk(p, st, ptab, j, first, last):
            nc.tensor.matmul(
                st["ata"],
                V[j][:, 192 * p : 192 * p + 128],
                ptab[:, 0:QB],
                start=first,
                stop=last,
            )
            nc.tensor.matmul(
                st["atb"],
                V[j][:, 192 * p + 64 : 192 * p + 192],
                ptab[:, QB : 2 * QB],
                start=first,
                stop=last,
            )

        def emit_normalize(p, tb, st):
            # out = at * (1/sum). reciprocal_approx_fast (custom-DVE)
            # requires base-partition-0 APs, so stage sumsA down to a
            # base-0 tile; mixed PSUM+SBUF operands may use different base
            # partitions, so the muls read the reciprocal tiles directly.
            ata, atb = st["ata"], st["atb"]
            combA = recp.tile([64, QB], f32, tag="combA", bufs=1)
            nc.vector.tensor_copy(combA, ata[64:128, :])
            rtA = recp.tile([64, QB], f32, tag="rtA", bufs=1)
            nc.vector.reciprocal_approx_fast(rtA, combA)
            rtB = recp.tile([64, QB], f32, tag="rtB", bufs=1)
            nc.vector.reciprocal_approx_fast(rtB, atb[0:64, :])
            ao = aoT[p][:, tb * QB : (tb + 1) * QB]
            nc.vector.tensor_mul(ao[0:64, :], ata[0:64, :], rtA)
            nc.vector.tensor_mul(ao[64:128, :], atb[64:128, :], rtB)

        def make_av_steps(p, tb, pts):
            st = {}

            def step(j):
                def run():
                    if j == 0:
                        st["ata"] = at_ps.tile([P, QB], f32, tag="at", name="ata")
                        st["atb"] = at_ps.tile([P, QB], f32, tag="at", name="atb")
                    emit_av_chunk(p, st, pts[j], j, j == 0, j == TG - 1)
                    if j == TG - 1:
                        emit_normalize(p, tb, st)

                return run

            return [step(j) for j in range(TG)]

        def emit_qk_chunk(KT, qa, qb, j, pts):
            # Per key chunk j, ONE 2-bank psum tile holds head A scores in
            # cols 0:QB and head B in QB:2QB, covered by ONE exp. Both QK
            # matmuls then wait on the same semaphore, co-dispatch, and run
            # concurrently in disjoint PE row groups (tile_position (0,0)
            # / (64,0) auto-derived from the 64-partition operands).
            stab = st_ps.tile([P, 2 * QB], f32, tag="st", name="stab")
            nc.tensor.matmul(stab[:, 0:QB], KT[0:64, j * P : (j + 1) * P], qa)
            nc.tensor.matmul(
                stab[:, QB : 2 * QB], KT[64:128, j * P : (j + 1) * P], qb
            )
            ptab = ptp.tile([P, 2 * QB], bf16, tag="pt", name="ptab")
            nc.scalar.activation(ptab, stab, AF.Exp)
            pts.append(ptab)

        # ---------------- per-pair: K proj, Q proj, attention ----------------
        kq = {}
        for _ in kq_proj_gen(0, kq):
            pass
        ow = {}
        lag = []
        pending.append(v_proj_gen(0))
        for p in range(NPAIR):
            KT, QT = kq["KT"], kq["QT"]
            kq = {}
            if p + 1 < NPAIR:
                if p == 0:
                    # pair 0: V fh0 must finish first (AV(0) needs it by
                    # the second block); kq(1) has until pair 1 starts
                    pending.append(kq_proj_gen(1, kq))
                else:
                    # next pair's K/Q proj is urgent: queue front
                    pending.insert(0, kq_proj_gen(p + 1, kq))
            if p == 1:
                pending.append(v_proj_gen(1))
            if p == NPAIR - 2:
                pending.append(bias_woT_gen(ow))

            if p < NPAIR - 1:
                for tb in range(NB):
                    qa = QT[0:64, tb * QB : (tb + 1) * QB]
                    qb = QT[64:128, tb * QB : (tb + 1) * QB]
                    pts = []
                    for j in range(TG):
                        if lag:
                            lag.pop(0)()
                        emit_qk_chunk(KT, qa, qb, j, pts)
                        pump(2)
                    assert not lag
                    lag = make_av_steps(p, tb, pts)
                # make sure the next pair's KT/QT is fully emitted
                while "QT" not in kq:
                    pump()
            else:
                # last pair: inline AV so out-proj can chase each block
                for tb in range(NB):
                    qa = QT[0:64, tb * QB : (tb + 1) * QB]
                    qb = QT[64:128, tb * QB : (tb + 1) * QB]
                    pts = []
                    for j in range(TG):
                        if lag:
                            lag.pop(0)()
                        emit_qk_chunk(KT, qa, qb, j, pts)
                        pump(2)
                    st = {
                        "ata": at_ps.tile([P, QB], f32, tag="at", name="ata"),
                        "atb": at_ps.tile([P, QB], f32, tag="at", name="atb"),
                    }
                    for j in range(TG):
                        emit_av_chunk(p, st, pts[j], j, j == 0, j == TG - 1)
                        if j % 4 == 3:
                            pump()
                    emit_normalize(p, tb, st)
                    # all pairs done for this query block: out-project it,
                    # overlapping the last pair's remaining attention work
                    drain()
                    pending.append(out_proj_gen(tb * 4, (tb + 1) * 4, ow))
                    if tb == NB - 1:
                        drain()


def _run(in_maps, trace=False):
    from concourse.bass_utils import run_bass_kernel_spmd

    if "nc" not in _CACHE:
        _CACHE["nc"] = _build()
    nc = _CACHE["nc"]
    return run_bass_kernel_spmd(
        nc, in_maps, core_ids=list(range(8)), trace=trace
    )


def _make_in_maps(x, w_qkv, w_out, b_out):
    x = np.ascontiguousarray(np.asarray(x, dtype=np.float32))
    w_qkv = np.ascontiguousarray(np.asarray(w_qkv, dtype=np.float32))
    w_out = np.ascontiguousarray(np.asarray(w_out, dtype=np.float32))
    b_out = np.ascontiguousarray(np.asarray(b_out, dtype=np.float32))
    in_maps = []
    for c in range(8):
        b, h = divmod(c, 2)
        xb = x[b]
        if h:
            xb = np.ascontiguousarray(
                np.concatenate([xb[TQ:], xb[:TQ]], axis=0)
            )
        in_maps.append(
            {"x": xb, "w_qkv": w_qkv, "w_out": w_out, "b_out": b_out}
        )
    return in_maps


def _gather(results):
    y = np.empty((B, N, C), dtype=np.float32)
    for c in range(8):
        b, h = divmod(c, 2)
        y[b, h * TQ : (h + 1) * TQ, :] = results[c]["y"]
    return y


def kernel(x, w_qkv, w_out, b_out):
    res = _run(_make_in_maps(x, w_qkv, w_out, b_out), trace=False)
    return _gather(res.results)



# revision 2
# speedup vs baseline: 1.3515x; 1.3515x over previous
"""Multi-head attention (qkv proj + softmax attention + out proj) on 8 trn2 cores.

Sharding: zero-collective. Core c handles batch b=c//2 and query-half h=c%2
(1024 queries). The host passes x[b] ROTATED by h*1024 tokens so that each
core's queries are always local tokens 0..1023 while K/V still cover all 2048
tokens (key order is irrelevant to softmax). Host concatenates the 8 output
slabs [1024, 1024] into [4, 2048, 1024].

Host-side marshaling (the key difference from the fp32-input variant): x and
all weights are pre-transposed to contraction-major layout, pre-cast to bf16,
and packed into the exact per-pair/per-group tile layouts the device wants.
The device therefore runs ZERO PE transposes and almost no DVE casts — the
PE issue pipe only carries real matmul work:
  1. QKV projection in bf16: QT/KT feature-major [d, t] so each 128-row tile
     holds a PAIR of heads (2 x 64 d-rows); V token-major with interleaved
     ones columns (fused softmax-sum rows come out of the AV matmul free).
  2. Attention per head-pair: QK^T into PSUM, softmax WITHOUT
     max-subtraction (scores ~N(0,1); fp32 exp is safe), exp on ScalarE
     straight out of PSUM, AV+sums fused, fast approximate reciprocal on DVE.
  3. Out-projection in bf16 + bias; y stored bf16 (host upcasts).
"""

import numpy as np

B, N, C = 4, 2048, 1024
H, D = 16, 64
P = 128
CG = C // P            # 8 contraction groups
TG = N // P            # 16 key-token chunks
TQ = N // 2            # 1024 queries per core
QB = 512               # query block (psum bank)
NB = TQ // QB          # 2
NPAIR = H // 2         # 8 head pairs
SCALE = 1.0 / np.sqrt(D).astype(np.float32)

_CACHE = {}


def _build():
    import concourse.tile as tile
    from concourse import bacc, mybir

    f32 = mybir.dt.float32
    bf16 = mybir.dt.bfloat16
    nc = bacc.Bacc(
        "TRN2", target_bir_lowering=False, debug=False, num_devices=8
    )
    xT_h = nc.dram_tensor("xT", [CG, P, N], bf16, kind="ExternalInput").ap()
    wk_h = nc.dram_tensor("wk", [NPAIR, P, CG, P], bf16, kind="ExternalInput").ap()
    wq_h = nc.dram_tensor("wq", [NPAIR, P, CG, P], bf16, kind="ExternalInput").ap()
    wv_h = nc.dram_tensor("wv", [P, CG, C], bf16, kind="ExternalInput").ap()
    wo_h = nc.dram_tensor("wo", [P, CG, C], bf16, kind="ExternalInput").ap()
    bout_h = nc.dram_tensor("b_out", [C], f32, kind="ExternalInput").ap()
    y_h = nc.dram_tensor("y", [TQ, C], bf16, kind="ExternalOutput").ap()

    with tile.TileContext(nc) as tc:
        _emit(tc, xT_h, wk_h, wq_h, wv_h, wo_h, bout_h, y_h)
    nc.compile()
    return nc


def _emit(tc, xT_h, wk_h, wq_h, wv_h, wo_h, bout_h, y_h):
    from contextlib import ExitStack

    from concourse import mybir

    f32 = mybir.dt.float32
    bf16 = mybir.dt.bfloat16
    AF = mybir.ActivationFunctionType
    nc = tc.nc

    with ExitStack() as ctx:
        # ---------------- pools ----------------
        const = ctx.enter_context(tc.tile_pool(name="const", bufs=1))
        big = ctx.enter_context(tc.tile_pool(name="big", bufs=1))
        ktp = ctx.enter_context(tc.tile_pool(name="ktp", bufs=2))
        qtp = ctx.enter_context(tc.tile_pool(name="qtp", bufs=2))
        wkp = ctx.enter_context(tc.tile_pool(name="wkp", bufs=2))
        wqp = ctx.enter_context(tc.tile_pool(name="wqp", bufs=2))
        ptp = ctx.enter_context(tc.tile_pool(name="ptp", bufs=17))
        recp = ctx.enter_context(tc.tile_pool(name="recp", bufs=2))
        outp = ctx.enter_context(tc.tile_pool(name="outp", bufs=2))
        ps_pool = ctx.enter_context(
            tc.tile_pool(name="ps_pool", bufs=2, space="PSUM")
        )
        st_ps = ctx.enter_context(
            tc.tile_pool(name="st_ps", bufs=2, space="PSUM")
        )
        at_ps = ctx.enter_context(
            tc.tile_pool(name="at_ps", bufs=2, space="PSUM")
        )

        # ---------------- constants ----------------
        onesrow = const.tile([1, P], f32)
        nc.gpsimd.memset(onesrow, 1.0)
        bl = const.tile([1, C], f32)
        nc.gpsimd.dma_start(bl, bout_h.unsqueeze(0))

        # ---------------- persistent tensors ----------------
        # V layout per pair p: cols [192p,192p+64)=V_h2p, [192p+64,+128)=ones,
        # [192p+128,+192)=V_h2p+1. The AV stationary operand for head A is
        # cols [192p,192p+128) = [V_A | ones] (output rows 0:64 = attention
        # out, rows 64:128 = softmax sums, replicated); for head B it is cols
        # [192p+64,192p+192) = [ones | V_B] (sums on rows 0:64, attention out
        # on rows 64:128). The ones block is shared between the two heads.
        xT = [big.tile([P, N], bf16, name=f"xT{g}") for g in range(CG)]
        V = [big.tile([P, 192 * NPAIR], bf16, name=f"V{i}") for i in range(TG)]
        aoT = [big.tile([P, TQ], bf16, name=f"aoT{g}") for g in range(CG)]
        wv = big.tile([P, CG, C], bf16, name="wv")
        wo = big.tile([P, CG, C], bf16, name="wo")

        # -------- input DMAs, in startup-critical order --------
        # Query-half columns of xT first (K proj blocks 0,1 + all of Q proj
        # touch only cols 0:1024), so the PE can start ~2 MB sooner.
        for g in range(CG):
            nc.sync.dma_start(xT[g][:, 0:TQ], xT_h[g][:, 0:TQ])
        for g in range(CG):
            nc.sync.dma_start(xT[g][:, TQ:N], xT_h[g][:, TQ:N])
        nc.sync.dma_start(wv, wv_h)
        nc.sync.dma_start(wo, wo_h)

        # V ones columns (needed before the first AV read, ~30us in).
        for i in range(TG):
            v3i = V[i].rearrange("p (q e) -> p q e", e=64)
            nc.gpsimd.memset(v3i[:, 1 : 3 * NPAIR : 3], 1.0)

        # ---- deferred-emission machinery: projection work for pair p+1 is
        # emitted in small quanta INTO pair p's attention emission, so the
        # Tile scheduler gives it priorities that interleave it into the
        # PE-idle slots of the (Scalar-exp-paced) attention phase instead of
        # serializing it at the pair boundary.
        def kq_proj_gen(p, out):
            # K pair p (w already contraction-major + bf16 from the host)
            wkt = wkp.tile([P, CG, P], bf16, tag="wk")
            nc.scalar.dma_start(wkt, wk_h[p])
            wqt = wqp.tile([P, CG, P], bf16, tag="wq")
            nc.scalar.dma_start(wqt, wq_h[p])
            KT = ktp.tile([P, N], bf16, tag="KT")
            yield
            for tbp in range(0, 4, 2):
                pss = [
                    ps_pool.tile([P, QB], f32, tag="ps", name=f"pjk{k}")
                    for k in range(2)
                ]
                for g in range(CG):
                    for k in range(2):
                        nc.tensor.matmul(
                            pss[k],
                            wkt[:, g, :],
                            xT[g][:, (tbp + k) * QB : (tbp + k + 1) * QB],
                            start=(g == 0),
                            stop=(g == CG - 1),
                        )
                    yield
                for k in range(2):
                    nc.vector.tensor_copy(
                        KT[:, (tbp + k) * QB : (tbp + k + 1) * QB], pss[k]
                    )
                yield
            out["KT"] = KT

            # Q pair p: queries are tokens 0..TQ-1 of the (rotated) local x.
            # Scale 1/sqrt(D) pre-folded into wq on the host.
            QT = qtp.tile([P, TQ], bf16, tag="QT")
            pss = [
                ps_pool.tile([P, QB], f32, tag="ps", name=f"pjq{k}")
                for k in range(2)
            ]
            for g in range(CG):
                for k in range(2):
                    nc.tensor.matmul(
                        pss[k],
                        wqt[:, g, :],
                        xT[g][:, k * QB : (k + 1) * QB],
                        start=(g == 0),
                        stop=(g == CG - 1),
                    )
                yield
            for k in range(2):
                nc.vector.tensor_copy(QT[:, k * QB : (k + 1) * QB], pss[k])
            out["QT"] = QT

        def v_proj_gen(fh):
            # V feature half fh: heads 8fh..8fh+7 (pairs 4fh..4fh+3).
            for i in range(TG):
                ps = ps_pool.tile([P, QB], f32, tag="ps")
                for g in range(CG):
                    nc.tensor.matmul(
                        ps,
                        xT[g][:, i * P : (i + 1) * P],
                        wv[:, g, fh * QB : (fh + 1) * QB],
                        start=(g == 0),
                        stop=(g == CG - 1),
                    )
                    if g % 4 == 3:
                        yield
                # psum cols = v features [512*fh, 512*(fh+1)) = heads
                # 8fh..8fh+7. Scatter per-head 64-col blocks into the
                # interleaved V layout: head h -> col 192*(h//2)+128*(h%2).
                ps3 = ps.rearrange("p (k e) -> p k e", e=64)
                v3 = V[i].rearrange("p (q e) -> p q e", e=64)
                b0 = 12 * fh
                nc.vector.tensor_copy(v3[:, b0 : b0 + 12 : 3], ps3[:, 0::2])
                nc.vector.tensor_copy(
                    v3[:, b0 + 2 : b0 + 12 : 3], ps3[:, 1::2]
                )
                yield

        def bias_gen(out):
            bias = big.tile([P, C], f32, name="bias")
            yield
            for hh in range(2):
                ps = ps_pool.tile([P, QB], f32, tag="ps")
                nc.tensor.matmul(
                    ps, onesrow, bl[0:1, hh * QB : (hh + 1) * QB]
                )
                nc.scalar.copy(bias[:, hh * QB : (hh + 1) * QB], ps)
                yield
            out["bias"] = bias

        def out_proj_gen(i0, i1, ow):
            for i in range(i0, i1):
                ob = outp.tile([P, C], bf16, tag="ob")
                for oh in range(2):
                    ps = ps_pool.tile([P, QB], f32, tag="ps")
                    for g in range(CG):
                        nc.tensor.matmul(
                            ps,
                            aoT[g][:, i * P : (i + 1) * P],
                            wo[:, g, oh * QB : (oh + 1) * QB],
                            start=(g == 0),
                            stop=(g == CG - 1),
                        )
                        if g % 4 == 3:
                            yield
                    nc.vector.tensor_add(
                        ob[:, oh * QB : (oh + 1) * QB],
                        ps,
                        ow["bias"][:, oh * QB : (oh + 1) * QB],
                    )
                nc.sync.dma_start(y_h[i * P : (i + 1) * P, :], ob)
                yield

        pending = []

        def pump(n=1):
            for _ in range(n):
                while pending:
                    try:
                        next(pending[0])
                        break
                    except StopIteration:
                        pending.pop(0)
                else:
                    return

        def drain():
            while pending:
                pump()

        # ---- lagged AV: each block's AV + normalize is emitted chunk-by-
        # chunk INSIDE the next block's QK loop, so the Scalar exp chain
        # never stalls behind an AV wall at block/pair boundaries.
        def emit_av_chunk(p, st, ptab, j, first, last):
            nc.tensor.matmul(
                st["ata"],
                V[j][:, 192 * p : 192 * p + 128],
                ptab[:, 0:QB],
                start=first,
                stop=last,
            )
            nc.tensor.matmul(
                st["atb"],
                V[j][:, 192 * p + 64 : 192 * p + 192],
                ptab[:, QB : 2 * QB],
                start=first,
                stop=last,
            )

        def emit_normalize(p, tb, st):
            # out = at * (1/sum). reciprocal_approx_fast (custom-DVE)
            # requires base-partition-0 APs, so stage sumsA down to a
            # base-0 tile; mixed PSUM+SBUF operands may use different base
            # partitions, so the muls read the reciprocal tiles directly.
            ata, atb = st["ata"], st["atb"]
            combA = recp.tile([64, QB], f32, tag="combA", bufs=1)
            nc.vector.tensor_copy(combA, ata[64:128, :])
            rtA = recp.tile([64, QB], f32, tag="rtA", bufs=1)
            nc.vector.reciprocal_approx_fast(rtA, combA)
            rtB = recp.tile([64, QB], f32, tag="rtB", bufs=1)
            nc.vector.reciprocal_approx_fast(rtB, atb[0:64, :])
            ao = aoT[p][:, tb * QB : (tb + 1) * QB]
            nc.vector.tensor_mul(ao[0:64, :], ata[0:64, :], rtA)
            nc.vector.tensor_mul(ao[64:128, :], atb[64:128, :], rtB)

        def make_av_steps(p, tb, pts):
            st = {}

            def step(j):
                def run():
                    if j == 0:
                        st["ata"] = at_ps.tile([P, QB], f32, tag="at", name="ata")
                        st["atb"] = at_ps.tile([P, QB], f32, tag="at", name="atb")
                    emit_av_chunk(p, st, pts[j], j, j == 0, j == TG - 1)
                    if j == TG - 1:
                        emit_normalize(p, tb, st)

                return run

            return [step(j) for j in range(TG)]

        def emit_qk_chunk(KT, qa, qb, j, pts):
            # Per key chunk j, ONE 2-bank psum tile holds head A scores in
            # cols 0:QB and head B in QB:2QB, covered by ONE exp. Both QK
            # matmuls then wait on the same semaphore, co-dispatch, and run
            # concurrently in disjoint PE row groups (tile_position (0,0)
            # / (64,0) auto-derived from the 64-partition operands).
            stab = st_ps.tile([P, 2 * QB], f32, tag="st", name="stab")
            nc.tensor.matmul(stab[:, 0:QB], KT[0:64, j * P : (j + 1) * P], qa)
            nc.tensor.matmul(
                stab[:, QB : 2 * QB], KT[64:128, j * P : (j + 1) * P], qb
            )
            ptab = ptp.tile([P, 2 * QB], bf16, tag="pt", name="ptab")
            nc.scalar.activation(ptab, stab, AF.Exp)
            pts.append(ptab)

        # ---------------- per-pair: K proj, Q proj, attention ----------------
        kq = {}
        for _ in kq_proj_gen(0, kq):
            pass
        ow = {}
        lag = []
        pending.append(bias_gen(ow))
        pending.append(v_proj_gen(0))
        for p in range(NPAIR):
            KT, QT = kq["KT"], kq["QT"]
            kq = {}
            if p + 1 < NPAIR:
                if p == 0:
                    # pair 0: V fh0 must finish first (AV(0) needs it by
                    # the second block); kq(1) has until pair 1 starts
                    pending.append(kq_proj_gen(1, kq))
                else:
                    # next pair's K/Q proj is urgent: queue front
                    pending.insert(0, kq_proj_gen(p + 1, kq))
            if p == 1:
                pending.append(v_proj_gen(1))

            if p < NPAIR - 1:
                for tb in range(NB):
                    qa = QT[0:64, tb * QB : (tb + 1) * QB]
                    qb = QT[64:128, tb * QB : (tb + 1) * QB]
                    pts = []
                    for j in range(TG):
                        if lag:
                            lag.pop(0)()
                        emit_qk_chunk(KT, qa, qb, j, pts)
                        pump(2)
                    assert not lag
                    lag = make_av_steps(p, tb, pts)
                # make sure the next pair's KT/QT is fully emitted
                while "QT" not in kq:
                    pump()
            else:
                # last pair: inline AV so out-proj can chase each block
                for tb in range(NB):
                    qa = QT[0:64, tb * QB : (tb + 1) * QB]
                    qb = QT[64:128, tb * QB : (tb + 1) * QB]
                    pts = []
                    for j in range(TG):
                        if lag:
                            lag.pop(0)()
                        emit_qk_chunk(KT, qa, qb, j, pts)
                        pump(2)
                    st = {
                        "ata": at_ps.tile([P, QB], f32, tag="at", name="ata"),
                        "atb": at_ps.tile([P, QB], f32, tag="at", name="atb"),
                    }
                    for j in range(TG):
                        emit_av_chunk(p, st, pts[j], j, j == 0, j == TG - 1)
                        if j % 4 == 3:
                            pump()
                    emit_normalize(p, tb, st)
                    # all pairs done for this query block: out-project it,
                    # overlapping the last pair's remaining attention work
                    drain()
                    pending.append(out_proj_gen(tb * 4, (tb + 1) * 4, ow))
                    if tb == NB - 1:
                        drain()


def _run(in_maps, trace=False):
    from concourse.bass_utils import run_bass_kernel_spmd

    if "nc" not in _CACHE:
        _CACHE["nc"] = _build()
    nc = _CACHE["nc"]
    return run_bass_kernel_spmd(
        nc, in_maps, core_ids=list(range(8)), trace=trace
    )


def _make_in_maps(x, w_qkv, w_out, b_out):
    import ml_dtypes

    bf16 = ml_dtypes.bfloat16
    x = np.asarray(x, dtype=np.float32)
    w_qkv = np.asarray(w_qkv, dtype=np.float32)
    w_out = np.asarray(w_out, dtype=np.float32)
    b_out = np.ascontiguousarray(np.asarray(b_out, dtype=np.float32))

    # Device-layout weight packs (shared by all 8 cores).
    # wk/wq: [pair, part(=cin%128), g(=cin//128), col(=feat within pair)]
    def kq_pack(wrows):
        a = wrows.T.reshape(CG, P, NPAIR, P)          # [g, r, pair, c]
        return np.ascontiguousarray(
            a.transpose(2, 1, 0, 3).astype(bf16)      # [pair, r, g, c]
        )

    wq = kq_pack(w_qkv[0:C] * SCALE)
    wk = kq_pack(w_qkv[C : 2 * C])
    # wv/wo: [part(=cin%128), g(=cin//128), feat]
    def vo_pack(wrows):
        a = wrows.T.reshape(CG, P, C)                 # [g, r, f]
        return np.ascontiguousarray(a.transpose(1, 0, 2).astype(bf16))

    wv = vo_pack(w_qkv[2 * C : 3 * C])
    wo = vo_pack(w_out)

    in_maps = []
    for c in range(8):
        b, h = divmod(c, 2)
        xb = x[b]
        if h:
            xb = np.concatenate([xb[TQ:], xb[:TQ]], axis=0)
        xT = np.ascontiguousarray(xb.T.astype(bf16)).reshape(CG, P, N)
        in_maps.append(
            {"xT": xT, "wk": wk, "wq": wq, "wv": wv, "wo": wo,
             "b_out": b_out}
        )
    return in_maps


def _gather(results):
    y = np.empty((B, N, C), dtype=np.float32)
    for c in range(8):
        b, h = divmod(c, 2)
        y[b, h * TQ : (h + 1) * TQ, :] = results[c]["y"].astype(np.float32)
    return y


def kernel(x, w_qkv, w_out, b_out):
    res = _run(_make_in_maps(x, w_qkv, w_out, b_out), trace=False)
    return _gather(res.results)
